# revision 23
# baseline (speedup 1.0000x reference)
"""Trainium2 Bass kernel for Mistral-style attention with an INVERTED band mask.

Reference semantics (S=2048, E=4096, H=32, KV=8, D=128, WINDOW=1024):
  q/k/v projections -> RoPE(q,k) -> GQA attention where positions with
  |i-j| < 1024 are masked OUT (attend only to far positions) -> softmax ->
  out projection.

Sharding (8 cores, tensor-parallel by GQA group):
  core c owns KV head c and Q heads 4c..4c+3. Column-parallel QKV,
  row-parallel O projection; the 8 fp16 partial outputs are summed on host.

On-device layout: everything transposed so matmuls contract on partitions.
  Host passes hidden^T, Wq^T/Wk^T/Wv^T slices, Wo^T slice, RoPE tables
  (transposed, sign-folded, fp16), and two 128x128 triangular masks for the
  blocks that straddle the |i-j|=1024 boundary.

Schedule notes (v2):
  - Input DMAs are interleaved with phase-1 matmuls (weights stream in per
    e-tile pair) so the PE starts ~2us in instead of after the full 29MB load.
  - Phase-1 chunks are processed in order [1,3,0,2] and attention chunks in
    [1,3,0,2] with the O projection shifted one chunk behind, so RoPE/exp/
    normalize latencies never gate the in-order PE stream.
  - Denominator = ones[128x128] @ exp(scores) -> [128,512] PSUM with the sum
    replicated across partitions; reciprocal on DVE (full tile, same cost as
    one row), then one tensor_tensor multiply. No gpsimd broadcast.
  - Score/psa/psD matmuls stream only the valid column range per key block
    (full-width block first with start=True so has_written covers the bank).
"""

import math
from contextlib import ExitStack

import numpy as np
import ml_dtypes

import concourse.bass as bass
import concourse.mybir as mybir
import concourse.tile as tile
from concourse import bacc
from concourse.bass_utils import run_bass_kernel_spmd

P = 128
S = 2048
E = 4096
D = 128
HPC = 4          # q heads per core
NE = E // P      # 32 e-tiles
NSCH = 4         # s-chunks of 512
SCH = S // NSCH  # 512
NST = S // P     # 16 s-tiles
NEO = 8          # output e-chunks of 512
SCALE = 1.0 / math.sqrt(D)
F16 = mybir.dt.float16
F32 = mybir.dt.float32
BF16 = mybir.dt.bfloat16

P1_ORDER = [1, 3, 0, 2]   # phase-1 chunk processing order
ATT_ORDER = [1, 3, 0, 2]  # attention chunk order (o-proj shifted one behind)


def _allowed_tiles(c):
    """For s-chunk c (query blocks bi=4c..4c+3), list (bj, lo, hi, mask, mpos):
    key tile bj is needed for query sub-tiles [lo, hi) (chunk-relative);
    mask in {None,'low','up'} applied at chunk-relative position mpos.
    Ordered with a full-width block first (for PSUM start=True coverage)."""
    out = []
    bis = range(4 * c, 4 * c + 4)
    for bj in range(NST):
        ok = [bi for bi in bis if abs(bi - bj) >= 8]
        if not ok:
            continue
        lo = min(ok) - 4 * c
        hi = max(ok) + 1 - 4 * c
        assert ok == list(range(lo + 4 * c, hi + 4 * c)), (c, bj, ok)
        mask, mpos = None, 0
        if bj - 8 in ok:
            mask, mpos = "low", bj - 8 - 4 * c
        elif bj + 8 in ok:
            mask, mpos = "up", bj + 8 - 4 * c
        out.append((bj, lo, hi, mask, mpos))
    out.sort(key=lambda t: (t[1] - t[2], t[0]))  # widest first
    assert out[0][1] == 0 and out[0][2] == 4, (c, out[0])
    return out


def build_nc():
    nc = bacc.Bacc("TRN2", target_bir_lowering=False, debug=False)
    hidT = nc.dram_tensor("hidT", (E, S), F16, kind="ExternalInput")
    wqT = nc.dram_tensor("wqT", (E, HPC * D), F16, kind="ExternalInput")
    wkT = nc.dram_tensor("wkT", (E, D), F16, kind="ExternalInput")
    wvT = nc.dram_tensor("wvT", (E, D), F16, kind="ExternalInput")
    woT = nc.dram_tensor("woT", (HPC * D, E), F16, kind="ExternalInput")
    cosT = nc.dram_tensor("cosT", (D, S), F16, kind="ExternalInput")
    sinT = nc.dram_tensor("sinT", (D, S), F16, kind="ExternalInput")
    # additive masks: -1e4 at banned positions of the boundary blocks, and an
    # identity used to add them to scores in PSUM via a 128-col matmul
    mlneg = nc.dram_tensor("mlneg", (P, P), F16, kind="ExternalInput")
    muneg = nc.dram_tensor("muneg", (P, P), F16, kind="ExternalInput")
    ident = nc.dram_tensor("ident", (P, P), F16, kind="ExternalInput")
    outd = nc.dram_tensor("out", (S, E), F16, kind="ExternalOutput")

    with tile.TileContext(nc) as tc, ExitStack() as ctx:
        const = ctx.enter_context(tc.tile_pool(name="const", bufs=1))

        wqT_r = wqT.rearrange("(eo p) d -> p eo d", p=P)
        wkT_r = wkT.rearrange("(eo p) d -> p eo d", p=P)
        wvT_r = wvT.rearrange("(eo p) d -> p eo d", p=P)
        hidT_r = hidT.rearrange("(eo p) s -> p eo s", p=P)
        woT_r = woT.rearrange("(ho p) e -> p ho e", p=P)

        NP = NE // 2  # e-tile pairs
        wq_t = [const.tile([P, 2, HPC * D], F16, name=f"wq{i}") for i in range(NP)]
        wk_t = [const.tile([P, 2, D], F16, name=f"wk{i}") for i in range(NP)]
        wv_t = [const.tile([P, 2, D], F16, name=f"wv{i}") for i in range(NP)]
        wo_t = [const.tile([P, E], F16, name=f"wo{h}") for h in range(HPC)]
        cos_sb = const.tile([P, S], F16)
        sin_sb = const.tile([P, S], F16)
        ml_sb = const.tile([P, P], F16)
        mu_sb = const.tile([P, P], F16)
        id_sb = const.tile([P, P], F16)
        ones_sb = const.tile([P, P], F16)
        nc.gpsimd.memset(ones_sb[:], 1.0)
        # Tiny dummy exp so the activation table load happens during the
        # initial DMA fill instead of gating the first attention block.
        warm = const.tile([1, 1], F32)
        nc.scalar.activation(
            warm[:], ones_sb[0:1, 0:1], mybir.ActivationFunctionType.Exp)

        qT_sb = const.tile([P, HPC, S], F16)     # Q^T per head [d, s]
        kT_sb = const.tile([P, S], F16)          # K^T [d, s]
        v_sb = const.tile([P, NST, D], F16)      # V [s-tile, d]
        attn_sb = const.tile([P, HPC, S], F16)   # attn_out^T per head [d, s]

        hidp = ctx.enter_context(tc.tile_pool(name="hid", bufs=4))
        rp = ctx.enter_context(tc.tile_pool(name="rope", bufs=2))

        def rope_drain(src_psum):
            raw = rp.tile([P, SCH], F16, tag="raw", bufs=6)
            nc.vector.tensor_copy(raw[:], src_psum)
            return raw

        def rope_apply(raw, dst_ap, c):
            rot = rp.tile([P, SCH], F16, tag="rot", bufs=2)
            nc.sync.dma_start(rot[0:64, :], raw[64:128, :])
            nc.sync.dma_start(rot[64:128, :], raw[0:64, :])
            t1 = rp.tile([P, SCH], F16, tag="t1", bufs=2)
            nc.vector.tensor_tensor(
                t1[:], raw[:], cos_sb[:, c * SCH:(c + 1) * SCH], mybir.AluOpType.mult)
            t2 = rp.tile([P, SCH], F16, tag="t2", bufs=2)
            nc.vector.tensor_tensor(
                t2[:], rot[:], sin_sb[:, c * SCH:(c + 1) * SCH], mybir.AluOpType.mult)
            nc.vector.tensor_tensor(dst_ap, t1[:], t2[:], mybir.AluOpType.add)

        # ---- Phase 1: QKV projections (+RoPE), inputs streamed in ----
        # Flat 64-step schedule (4 chunks x 16 e-tile pairs) with fixed DMA
        # lookahead so hid/weight transfers stay just ahead of the PE and
        # phase-boundary stalls vanish. Lower-priority inputs (cos/sin,
        # masks, wo) are slotted in after the critical stream.
        steps = [(c, i) for c in P1_ORDER for i in range(NP)]
        ht_tiles = {}

        def issue_ht(n):
            # scalar-queue HWDGE: the Sync DGE's ~0.7us/issue was co-critical
            # with compute in phase 1 when it carried both weights and hid
            c, i = steps[n]
            t = hidp.tile([P, 2, SCH], F16, tag="hid", bufs=6)
            nc.scalar.dma_start(
                t[:], hidT_r[:, 2 * i:2 * i + 2, c * SCH:(c + 1) * SCH])
            ht_tiles[n] = t

        def issue_w(i):
            nc.sync.dma_start(wq_t[i][:], wqT_r[:, 2 * i:2 * i + 2, :])
            nc.sync.dma_start(wk_t[i][:], wkT_r[:, 2 * i:2 * i + 2, :])
            nc.sync.dma_start(wv_t[i][:], wvT_r[:, 2 * i:2 * i + 2, :])

        with tc.tile_pool(name="p1psum", bufs=1, space="PSUM") as p1, \
             tc.tile_pool(name="p1kv", bufs=2, space="PSUM") as p1kv:
            issue_w(0)
            issue_ht(0)
            issue_ht(1)
            psq = psk = psvT = None
            for n, (c, i) in enumerate(steps):
                if i == 0:
                    psq = p1.tile([P, HPC, SCH], F32, tag="psq")   # 4 banks
                    psk = p1kv.tile([P, SCH], F32, tag="psk")      # 2 banks
                    psvT = p1kv.tile([P, SCH], F32, tag="psv")     # 2 banks
                ht = ht_tiles.pop(n)
                for t in range(2):
                    e = 2 * i + t
                    st = (e == 0)
                    sp = (e == NE - 1)
                    for h in range(HPC):
                        nc.tensor.matmul(
                            psq[:, h, :], wq_t[i][:, t, h * D:(h + 1) * D],
                            ht[:, t, :], start=st, stop=sp)
                    nc.tensor.matmul(
                        psk[:], wk_t[i][:, t, :], ht[:, t, :], start=st, stop=sp)
                    nc.tensor.matmul(
                        psvT[:], wv_t[i][:, t, :], ht[:, t, :], start=st, stop=sp)
                if n + 1 < NP:
                    issue_w(n + 1)
                if n + 2 < len(steps):
                    issue_ht(n + 2)
                if n == 8:
                    nc.sync.dma_start(cos_sb[:], cosT[:])
                    nc.sync.dma_start(sin_sb[:], sinT[:])
                elif n == 20:
                    nc.sync.dma_start(ml_sb[:], mlneg[:])
                    nc.sync.dma_start(mu_sb[:], muneg[:])
                    nc.sync.dma_start(id_sb[:], ident[:])
                elif 24 <= n < 24 + HPC:
                    nc.sync.dma_start(wo_t[n - 24][:], woT_r[:, n - 24, :])
                if i == NP - 1:
                    vstage = rp.tile([P, SCH], F16, tag="vstage", bufs=2)
                    nc.scalar.copy(vstage[:], psvT[:])
                    nc.sync.dma_start_transpose(
                        v_sb[:, c * 4:(c + 1) * 4, :], vstage[:])
                    kraw = rope_drain(psk[:])
                    # single wide CAST frees all 4 psq banks in one op, so
                    # the next chunk (or attention) reuses them ~2us sooner
                    qraw4 = rp.tile([P, HPC, SCH], F16, tag="qraw4", bufs=2)
                    nc.vector.tensor_copy(qraw4[:], psq[:])
                    rope_apply(kraw, kT_sb[:, c * SCH:(c + 1) * SCH], c)
                    for h in range(HPC):
                        rope_apply(qraw4[:, h, :],
                                   qT_sb[:, h, c * SCH:(c + 1) * SCH], c)

        # ---- Phase 2+3: attention, O projection one chunk behind ----
        ep = ctx.enter_context(tc.tile_pool(name="expp", bufs=3))
        np_pool = ctx.enter_context(tc.tile_pool(name="normp", bufs=2))
        osp = ctx.enter_context(tc.tile_pool(name="ostage", bufs=4))
        ap = ctx.enter_context(tc.tile_pool(name="apsum", bufs=2, space="PSUM"))

        def attention(c, h):
            blocks = _allowed_tiles(c)
            nblk = len(blocks)
            if True:
                psa = ap.tile([P, SCH], F32, tag="psa")
                psD = ap.tile([P, SCH], F32, tag="psd")
                ets = [None] * nblk
                # software-pipeline: scores/exp run one block ahead of psa/psD
                for idx in range(nblk + 1):
                    if idx < nblk:
                        bj, lo, hi, mask, mpos = blocks[idx]
                        n = (hi - lo) * P
                        pss = ap.tile([P, SCH], F32, tag="pss")
                        nc.tensor.matmul(
                            pss[:, :n],
                            kT_sb[:, bj * P:(bj + 1) * P],
                            qT_sb[:, h, c * SCH + lo * P: c * SCH + hi * P],
                            start=True, stop=(mask is None))
                        if mask is not None:
                            # add -1e4 at banned positions on the PE: identity
                            # stationary, pre-scaled triangle moving -> exp
                            # underflows to exact 0, no DVE op in the chain
                            m_sb = ml_sb if mask == "low" else mu_sb
                            mp = (mpos - lo) * P
                            nc.tensor.matmul(
                                pss[:, mp:mp + P], id_sb[:], m_sb[:],
                                start=False, stop=True)
                        et = ep.tile([P, SCH], BF16, tag="exp")
                        nc.scalar.activation(
                            et[:, lo * P:hi * P], pss[:, :n],
                            mybir.ActivationFunctionType.Exp, scale=SCALE)
                        ets[idx] = et
                    if idx >= 1:
                        bj, lo, hi, _, _ = blocks[idx - 1]
                        et = ets[idx - 1]
                        nc.tensor.matmul(
                            psa[:, lo * P:hi * P], v_sb[:, bj, :],
                            et[:, lo * P:hi * P],
                            start=(idx == 1), stop=(idx == nblk))
                        nc.tensor.matmul(
                            psD[:, lo * P:hi * P], ones_sb[:],
                            et[:, lo * P:hi * P],
                            start=(idx == 1), stop=(idx == nblk))
                invD = np_pool.tile([P, SCH], F32, tag="invd")
                nc.vector.reciprocal(invD[:], psD[:])
                nc.vector.tensor_tensor(
                    attn_sb[:, h, c * SCH:(c + 1) * SCH], psa[:], invD[:],
                    mybir.AluOpType.mult)

        def o_proj_tile(st):
            orow = osp.tile([P, E], F16, tag="orow", bufs=2)
            for eo in range(NEO):
                pso = ap.tile([P, SCH], F32, tag="pso")
                for h in range(HPC):
                    nc.tensor.matmul(
                        pso[:],
                        attn_sb[:, h, st * P:(st + 1) * P],
                        wo_t[h][:, eo * SCH:(eo + 1) * SCH],
                        start=(h == 0), stop=(h == HPC - 1))
                if eo % 2 == 0:
                    nc.scalar.copy(orow[:, eo * SCH:(eo + 1) * SCH], pso[:])
                else:
                    nc.vector.tensor_copy(
                        orow[:, eo * SCH:(eo + 1) * SCH], pso[:])
                if eo == NEO // 2 - 1:
                    nc.sync.dma_start(
                        outd[st * P:(st + 1) * P, :E // 2],
                        orow[:, :E // 2])
            nc.sync.dma_start(
                outd[st * P:(st + 1) * P, E // 2:], orow[:, E // 2:])

        # attention heads of chunk c interleave 1:1 with O-projection
        # s-tiles of the previous chunk, so exp/drain work alternates in
        # every engine queue instead of batching up behind it
        prev = None
        for c in ATT_ORDER:
            for h in range(HPC):
                attention(c, h)
                if prev is not None:
                    o_proj_tile(4 * prev + h)
            prev = c
        for h in range(HPC):
            o_proj_tile(4 * prev + h)
    nc.compile()
    return nc


_NC_CACHE = {}


def get_nc():
    if "nc" not in _NC_CACHE:
        _NC_CACHE["nc"] = build_nc()
    return _NC_CACHE["nc"]


def make_in_maps(hidden_states, Wq, Wk, Wv, Wo):
    hid = np.asarray(hidden_states).reshape(S, E)
    hidT16 = np.ascontiguousarray(hid.T).astype(np.float16)

    inv = 1.0 / (10000.0 ** (np.arange(0, D, 2, dtype=np.float64) / D))
    t = np.arange(S, dtype=np.float64)
    fr = np.outer(t, inv)                      # [S, 64]
    emb = np.concatenate([fr, fr], axis=1)     # [S, 128]
    cosT = np.ascontiguousarray(np.cos(emb).T).astype(np.float16)
    sinT = np.ascontiguousarray(np.sin(emb).T).astype(np.float16)
    sinT[:64] *= -1.0                          # rotate_half sign fold

    jj = np.arange(P)[:, None]
    ii = np.arange(P)[None, :]
    # additive -1e4 at banned positions (block bj-bi=8 keeps j-i>=1024;
    # block bi-bj=8 keeps i-j>=1024)
    mlneg = (-1e4 * (jj < ii)).astype(np.float16)
    muneg = (-1e4 * (ii < jj)).astype(np.float16)
    ident = np.eye(P, dtype=np.float16)

    in_maps = []
    for c in range(8):
        qsl = slice(c * 512, (c + 1) * 512)
        ksl = slice(c * 128, (c + 1) * 128)
        in_maps.append({
            "hidT": hidT16,
            "wqT": np.ascontiguousarray(Wq[qsl].T).astype(np.float16),
            "wkT": np.ascontiguousarray(Wk[ksl].T).astype(np.float16),
            "wvT": np.ascontiguousarray(Wv[ksl].T).astype(np.float16),
            "woT": np.ascontiguousarray(Wo[:, qsl].T).astype(np.float16),
            "cosT": cosT,
            "sinT": sinT,
            "mlneg": mlneg,
            "muneg": muneg,
            "ident": ident,
        })
    return in_maps


def run(in_maps, **kwargs):
    nc = get_nc()
    return run_bass_kernel_spmd(nc, in_maps, core_ids=list(range(8)), **kwargs)


def kernel(hidden_states, Wq, Wk, Wv, Wo):
    in_maps = make_in_maps(hidden_states, Wq, Wk, Wv, Wo)
    res = run(in_maps)
    out = np.zeros((S, E), dtype=np.float32)
    for r in res.results:
        out += r["out"].astype(np.float32)
    return out.reshape(1, S, E)


# revision 26
# speedup vs baseline: 1.0388x; 1.0388x over previous
"""Trainium2 Bass kernel for Mistral-style attention with an INVERTED band mask.

Reference semantics (S=2048, E=4096, H=32, KV=8, D=128, WINDOW=1024):
  q/k/v projections -> RoPE(q,k) -> GQA attention where positions with
  |i-j| < 1024 are masked OUT (attend only to far positions) -> softmax ->
  out projection.

Sharding (8 cores, tensor-parallel by GQA group):
  core c owns KV head c and Q heads 4c..4c+3. Column-parallel QKV,
  row-parallel O projection; the 8 fp16 partial outputs are summed on host.

On-device layout: everything transposed so matmuls contract on partitions.
  Host passes hidden^T, Wq^T/Wk^T/Wv^T slices, Wo^T slice, RoPE tables
  (transposed, sign-folded, fp16), and two 128x128 triangular masks for the
  blocks that straddle the |i-j|=1024 boundary.

Schedule notes (v2):
  - Input DMAs are interleaved with phase-1 matmuls (weights stream in per
    e-tile pair) so the PE starts ~2us in instead of after the full 29MB load.
  - Phase-1 chunks are processed in order [1,3,0,2] and attention chunks in
    [1,3,0,2] with the O projection shifted one chunk behind, so RoPE/exp/
    normalize latencies never gate the in-order PE stream.
  - Denominator = ones[128x128] @ exp(scores) -> [128,512] PSUM with the sum
    replicated across partitions; reciprocal on DVE (full tile, same cost as
    one row), then one tensor_tensor multiply. No gpsimd broadcast.
  - Score/psa/psD matmuls stream only the valid column range per key block
    (full-width block first with start=True so has_written covers the bank).
"""

import math
from contextlib import ExitStack

import numpy as np
import ml_dtypes

import concourse.bass as bass
import concourse.mybir as mybir
import concourse.tile as tile
from concourse import bacc
from concourse.bass_utils import run_bass_kernel_spmd

P = 128
S = 2048
E = 4096
D = 128
HPC = 4          # q heads per core
NE = E // P      # 32 e-tiles
NSCH = 4         # s-chunks of 512
SCH = S // NSCH  # 512
NST = S // P     # 16 s-tiles
NEO = 8          # output e-chunks of 512
SCALE = 1.0 / math.sqrt(D)
F16 = mybir.dt.float16
F32 = mybir.dt.float32
BF16 = mybir.dt.bfloat16

P1_ORDER = [1, 3, 0, 2]   # phase-1 chunk processing order
ATT_ORDER = [1, 3, 0, 2]  # attention chunk order (o-proj shifted one behind)


def _allowed_tiles(c):
    """For s-chunk c (query blocks bi=4c..4c+3), list (bj, lo, hi, mask, mpos):
    key tile bj is needed for query sub-tiles [lo, hi) (chunk-relative);
    mask in {None,'low','up'} applied at chunk-relative position mpos.
    Ordered with a full-width block first (for PSUM start=True coverage)."""
    out = []
    bis = range(4 * c, 4 * c + 4)
    for bj in range(NST):
        ok = [bi for bi in bis if abs(bi - bj) >= 8]
        if not ok:
            continue
        lo = min(ok) - 4 * c
        hi = max(ok) + 1 - 4 * c
        assert ok == list(range(lo + 4 * c, hi + 4 * c)), (c, bj, ok)
        mask, mpos = None, 0
        if bj - 8 in ok:
            mask, mpos = "low", bj - 8 - 4 * c
        elif bj + 8 in ok:
            mask, mpos = "up", bj + 8 - 4 * c
        out.append((bj, lo, hi, mask, mpos))
    out.sort(key=lambda t: (t[1] - t[2], t[0]))  # widest first
    assert out[0][1] == 0 and out[0][2] == 4, (c, out[0])
    return out


def build_nc():
    nc = bacc.Bacc("TRN2", target_bir_lowering=False, debug=False)
    hidT = nc.dram_tensor("hidT", (E, S), F16, kind="ExternalInput")
    wqT = nc.dram_tensor("wqT", (E, HPC * D), F16, kind="ExternalInput")
    wkT = nc.dram_tensor("wkT", (E, D), F16, kind="ExternalInput")
    wvT = nc.dram_tensor("wvT", (E, D), F16, kind="ExternalInput")
    woT = nc.dram_tensor("woT", (HPC * D, E), F16, kind="ExternalInput")
    cosT = nc.dram_tensor("cosT", (D, S), F16, kind="ExternalInput")
    sinT = nc.dram_tensor("sinT", (D, S), F16, kind="ExternalInput")
    # additive masks: -1e4 at banned positions of the boundary blocks, and an
    # identity used to add them to scores in PSUM via a 128-col matmul
    mlneg = nc.dram_tensor("mlneg", (P, P), F16, kind="ExternalInput")
    muneg = nc.dram_tensor("muneg", (P, P), F16, kind="ExternalInput")
    ident = nc.dram_tensor("ident", (P, P), F16, kind="ExternalInput")
    outd = nc.dram_tensor("out", (S, E), F16, kind="ExternalOutput")

    with tile.TileContext(nc) as tc, ExitStack() as ctx:
        const = ctx.enter_context(tc.tile_pool(name="const", bufs=1))

        wqT_r = wqT.rearrange("(eo p) d -> p eo d", p=P)
        wkT_r = wkT.rearrange("(eo p) d -> p eo d", p=P)
        wvT_r = wvT.rearrange("(eo p) d -> p eo d", p=P)
        hidT_r = hidT.rearrange("(eo p) s -> p eo s", p=P)
        woT_r = woT.rearrange("(ho p) e -> p ho e", p=P)

        NP = NE // 2  # e-tile pairs
        wq_t = [const.tile([P, 2, HPC * D], F16, name=f"wq{i}") for i in range(NP)]
        wk_t = [const.tile([P, 2, D], F16, name=f"wk{i}") for i in range(NP)]
        wv_t = [const.tile([P, 2, D], F16, name=f"wv{i}") for i in range(NP)]
        wo_t = [const.tile([P, E], F16, name=f"wo{h}") for h in range(HPC)]
        cos_sb = const.tile([P, S], F16)
        sin_sb = const.tile([P, S], F16)
        ml_sb = const.tile([P, P], F16)
        mu_sb = const.tile([P, P], F16)
        id_sb = const.tile([P, P], F16)
        ones_sb = const.tile([P, P], F16)
        nc.gpsimd.memset(ones_sb[:], 1.0)
        # Tiny dummy exp so the activation table load happens during the
        # initial DMA fill instead of gating the first attention block.
        warm = const.tile([1, 1], F32)
        nc.scalar.activation(
            warm[:], ones_sb[0:1, 0:1], mybir.ActivationFunctionType.Exp)

        qT_sb = const.tile([P, HPC, S], F16)     # Q^T per head [d, s]
        kT_sb = const.tile([P, S], F16)          # K^T [d, s]
        v_sb = const.tile([P, NST, D], F16)      # V [s-tile, d]
        attn_sb = const.tile([P, HPC, S], F16)   # attn_out^T per head [d, s]

        hidp = ctx.enter_context(tc.tile_pool(name="hid", bufs=4))
        rp = ctx.enter_context(tc.tile_pool(name="rope", bufs=2))

        def rope_drain(src_psum):
            raw = rp.tile([P, SCH], F16, tag="raw", bufs=6)
            nc.vector.tensor_copy(raw[:], src_psum)
            return raw

        def rope_apply(raw, dst_ap, c):
            rot = rp.tile([P, SCH], F16, tag="rot", bufs=2)
            nc.sync.dma_start(rot[0:64, :], raw[64:128, :])
            nc.sync.dma_start(rot[64:128, :], raw[0:64, :])
            t1 = rp.tile([P, SCH], F16, tag="t1", bufs=2)
            nc.vector.tensor_tensor(
                t1[:], raw[:], cos_sb[:, c * SCH:(c + 1) * SCH], mybir.AluOpType.mult)
            t2 = rp.tile([P, SCH], F16, tag="t2", bufs=2)
            nc.vector.tensor_tensor(
                t2[:], rot[:], sin_sb[:, c * SCH:(c + 1) * SCH], mybir.AluOpType.mult)
            nc.vector.tensor_tensor(dst_ap, t1[:], t2[:], mybir.AluOpType.add)

        # ---- Phase 1: QKV projections (+RoPE), inputs streamed in ----
        # Flat 64-step schedule (4 chunks x 16 e-tile pairs) with fixed DMA
        # lookahead so hid/weight transfers stay just ahead of the PE and
        # phase-boundary stalls vanish. Lower-priority inputs (cos/sin,
        # masks, wo) are slotted in after the critical stream.
        steps = [(c, i) for c in P1_ORDER for i in range(NP)]
        ht_tiles = {}

        def issue_ht(n):
            # scalar-queue HWDGE: the Sync DGE's ~0.7us/issue was co-critical
            # with compute in phase 1 when it carried both weights and hid
            c, i = steps[n]
            t = hidp.tile([P, 2, SCH], F16, tag="hid", bufs=6)
            nc.scalar.dma_start(
                t[:], hidT_r[:, 2 * i:2 * i + 2, c * SCH:(c + 1) * SCH])
            ht_tiles[n] = t

        def issue_w(i):
            nc.sync.dma_start(wq_t[i][:], wqT_r[:, 2 * i:2 * i + 2, :])
            nc.sync.dma_start(wk_t[i][:], wkT_r[:, 2 * i:2 * i + 2, :])
            nc.sync.dma_start(wv_t[i][:], wvT_r[:, 2 * i:2 * i + 2, :])

        with tc.tile_pool(name="p1psum", bufs=1, space="PSUM") as p1, \
             tc.tile_pool(name="p1kv", bufs=2, space="PSUM") as p1kv:
            issue_w(0)
            issue_ht(0)
            issue_ht(1)
            psq = psk = psvT = None
            for n, (c, i) in enumerate(steps):
                if i == 0:
                    psq = p1.tile([P, HPC, SCH], F32, tag="psq")   # 4 banks
                    psk = p1kv.tile([P, SCH], F32, tag="psk")      # 2 banks
                    psvT = p1kv.tile([P, SCH], F32, tag="psv")     # 2 banks
                ht = ht_tiles.pop(n)
                for t in range(2):
                    e = 2 * i + t
                    st = (e == 0)
                    sp = (e == NE - 1)
                    for h in range(HPC):
                        nc.tensor.matmul(
                            psq[:, h, :], wq_t[i][:, t, h * D:(h + 1) * D],
                            ht[:, t, :], start=st, stop=sp)
                    nc.tensor.matmul(
                        psk[:], wk_t[i][:, t, :], ht[:, t, :], start=st, stop=sp)
                    nc.tensor.matmul(
                        psvT[:], wv_t[i][:, t, :], ht[:, t, :], start=st, stop=sp)
                if n + 1 < NP:
                    issue_w(n + 1)
                if n + 2 < len(steps):
                    issue_ht(n + 2)
                if n == 8:
                    nc.sync.dma_start(cos_sb[:], cosT[:])
                    nc.sync.dma_start(sin_sb[:], sinT[:])
                elif n == 20:
                    nc.sync.dma_start(ml_sb[:], mlneg[:])
                    nc.sync.dma_start(mu_sb[:], muneg[:])
                    nc.sync.dma_start(id_sb[:], ident[:])
                elif 24 <= n < 24 + HPC:
                    nc.sync.dma_start(wo_t[n - 24][:], woT_r[:, n - 24, :])
                if i == NP - 1:
                    vstage = rp.tile([P, SCH], F16, tag="vstage", bufs=2)
                    nc.scalar.copy(vstage[:], psvT[:])
                    nc.sync.dma_start_transpose(
                        v_sb[:, c * 4:(c + 1) * 4, :], vstage[:])
                    kraw = rope_drain(psk[:])
                    # single wide CAST frees all 4 psq banks in one op, so
                    # the next chunk (or attention) reuses them ~2us sooner
                    qraw4 = rp.tile([P, HPC, SCH], F16, tag="qraw4", bufs=2)
                    nc.vector.tensor_copy(qraw4[:], psq[:])
                    rope_apply(kraw, kT_sb[:, c * SCH:(c + 1) * SCH], c)
                    for h in range(HPC):
                        rope_apply(qraw4[:, h, :],
                                   qT_sb[:, h, c * SCH:(c + 1) * SCH], c)

        # ---- Phase 2+3: attention, O projection one chunk behind ----
        ep = ctx.enter_context(tc.tile_pool(name="expp", bufs=3))
        np_pool = ctx.enter_context(tc.tile_pool(name="normp", bufs=2))
        osp = ctx.enter_context(tc.tile_pool(name="ostage", bufs=4))
        ap = ctx.enter_context(tc.tile_pool(name="apsum", bufs=2, space="PSUM"))

        def attention(c, h):
            blocks = _allowed_tiles(c)
            nblk = len(blocks)
            if True:
                psa = ap.tile([P, SCH], F32, tag="psa")
                psD = ap.tile([P, SCH], F32, tag="psd")
                ets = [None] * nblk
                # software-pipeline: scores/exp run one block ahead of psa/psD
                for idx in range(nblk + 1):
                    if idx < nblk:
                        bj, lo, hi, mask, mpos = blocks[idx]
                        n = (hi - lo) * P
                        pss = ap.tile([P, SCH], F32, tag="pss")
                        nc.tensor.matmul(
                            pss[:, :n],
                            kT_sb[:, bj * P:(bj + 1) * P],
                            qT_sb[:, h, c * SCH + lo * P: c * SCH + hi * P],
                            start=True, stop=(mask is None))
                        if mask is not None:
                            # add -1e4 at banned positions on the PE: identity
                            # stationary, pre-scaled triangle moving -> exp
                            # underflows to exact 0, no DVE op in the chain
                            m_sb = ml_sb if mask == "low" else mu_sb
                            mp = (mpos - lo) * P
                            nc.tensor.matmul(
                                pss[:, mp:mp + P], id_sb[:], m_sb[:],
                                start=False, stop=True)
                        et = ep.tile([P, SCH], BF16, tag="exp")
                        nc.scalar.activation(
                            et[:, lo * P:hi * P], pss[:, :n],
                            mybir.ActivationFunctionType.Exp, scale=SCALE)
                        ets[idx] = et
                    if idx >= 1:
                        bj, lo, hi, _, _ = blocks[idx - 1]
                        et = ets[idx - 1]
                        nc.tensor.matmul(
                            psa[:, lo * P:hi * P], v_sb[:, bj, :],
                            et[:, lo * P:hi * P],
                            start=(idx == 1), stop=(idx == nblk))
                        nc.tensor.matmul(
                            psD[:, lo * P:hi * P], ones_sb[:],
                            et[:, lo * P:hi * P],
                            start=(idx == 1), stop=(idx == nblk))
                return psa, psD

        def normalize(c, h, psa, psD):
            invD = np_pool.tile([P, SCH], F32, tag="invd")
            nc.vector.reciprocal(invD[:], psD[:])
            nc.vector.tensor_tensor(
                attn_sb[:, h, c * SCH:(c + 1) * SCH], psa[:], invD[:],
                mybir.AluOpType.mult)

        def o_proj_tile(st):
            orow = osp.tile([P, E], F16, tag="orow", bufs=2)
            for eo in range(NEO):
                pso = ap.tile([P, SCH], F32, tag="pso")
                for h in range(HPC):
                    nc.tensor.matmul(
                        pso[:],
                        attn_sb[:, h, st * P:(st + 1) * P],
                        wo_t[h][:, eo * SCH:(eo + 1) * SCH],
                        start=(h == 0), stop=(h == HPC - 1))
                nc.vector.tensor_copy(
                    orow[:, eo * SCH:(eo + 1) * SCH], pso[:])
                if eo == NEO // 2 - 1:
                    nc.sync.dma_start(
                        outd[st * P:(st + 1) * P, :E // 2],
                        orow[:, :E // 2])
            nc.sync.dma_start(
                outd[st * P:(st + 1) * P, E // 2:], orow[:, E // 2:])

        # attention heads of chunk c interleave 1:1 with O-projection
        # s-tiles of the previous chunk. Exps (Scalar) gate psa matmuls and
        # drains (Vector) gate pso reuse, so each engine carries only its
        # PE-gating op stream; recip+mult trail at the end of each pair
        # (they only gate the NEXT chunk's O projection).
        prev = None
        for c in ATT_ORDER:
            for h in range(HPC):
                psa, psD = attention(c, h)
                if prev is not None:
                    o_proj_tile(4 * prev + h)
                normalize(c, h, psa, psD)
            prev = c
        for h in range(HPC):
            o_proj_tile(4 * prev + h)
    nc.compile()
    return nc


_NC_CACHE = {}


def get_nc():
    if "nc" not in _NC_CACHE:
        _NC_CACHE["nc"] = build_nc()
    return _NC_CACHE["nc"]


def make_in_maps(hidden_states, Wq, Wk, Wv, Wo):
    hid = np.asarray(hidden_states).reshape(S, E)
    hidT16 = np.ascontiguousarray(hid.T).astype(np.float16)

    inv = 1.0 / (10000.0 ** (np.arange(0, D, 2, dtype=np.float64) / D))
    t = np.arange(S, dtype=np.float64)
    fr = np.outer(t, inv)                      # [S, 64]
    emb = np.concatenate([fr, fr], axis=1)     # [S, 128]
    cosT = np.ascontiguousarray(np.cos(emb).T).astype(np.float16)
    sinT = np.ascontiguousarray(np.sin(emb).T).astype(np.float16)
    sinT[:64] *= -1.0                          # rotate_half sign fold

    jj = np.arange(P)[:, None]
    ii = np.arange(P)[None, :]
    # additive -1e4 at banned positions (block bj-bi=8 keeps j-i>=1024;
    # block bi-bj=8 keeps i-j>=1024)
    mlneg = (-1e4 * (jj < ii)).astype(np.float16)
    muneg = (-1e4 * (ii < jj)).astype(np.float16)
    ident = np.eye(P, dtype=np.float16)

    in_maps = []
    for c in range(8):
        qsl = slice(c * 512, (c + 1) * 512)
        ksl = slice(c * 128, (c + 1) * 128)
        in_maps.append({
            "hidT": hidT16,
            "wqT": np.ascontiguousarray(Wq[qsl].T).astype(np.float16),
            "wkT": np.ascontiguousarray(Wk[ksl].T).astype(np.float16),
            "wvT": np.ascontiguousarray(Wv[ksl].T).astype(np.float16),
            "woT": np.ascontiguousarray(Wo[:, qsl].T).astype(np.float16),
            "cosT": cosT,
            "sinT": sinT,
            "mlneg": mlneg,
            "muneg": muneg,
            "ident": ident,
        })
    return in_maps


def run(in_maps, **kwargs):
    nc = get_nc()
    return run_bass_kernel_spmd(nc, in_maps, core_ids=list(range(8)), **kwargs)


def kernel(hidden_states, Wq, Wk, Wv, Wo):
    in_maps = make_in_maps(hidden_states, Wq, Wk, Wv, Wo)
    res = run(in_maps)
    out = np.zeros((S, E), dtype=np.float32)
    for r in res.results:
        out += r["out"].astype(np.float32)
    return out.reshape(1, S, E)


# revision 29
# speedup vs baseline: 1.0585x; 1.0189x over previous
"""Trainium2 Bass kernel for Mistral-style attention with an INVERTED band mask.

Reference semantics (S=2048, E=4096, H=32, KV=8, D=128, WINDOW=1024):
  q/k/v projections -> RoPE(q,k) -> GQA attention where positions with
  |i-j| < 1024 are masked OUT (attend only to far positions) -> softmax ->
  out projection.

Sharding (8 cores, tensor-parallel by GQA group):
  core c owns KV head c and Q heads 4c..4c+3. Column-parallel QKV,
  row-parallel O projection; the 8 fp16 partial outputs are summed on host.

On-device layout: everything transposed so matmuls contract on partitions.
  Host passes hidden^T, Wq^T/Wk^T/Wv^T slices, Wo^T slice, RoPE tables
  (transposed, sign-folded, fp16), and two 128x128 triangular masks for the
  blocks that straddle the |i-j|=1024 boundary.

Schedule notes (v2):
  - Input DMAs are interleaved with phase-1 matmuls (weights stream in per
    e-tile pair) so the PE starts ~2us in instead of after the full 29MB load.
  - Phase-1 chunks are processed in order [1,3,0,2] and attention chunks in
    [1,3,0,2] with the O projection shifted one chunk behind, so RoPE/exp/
    normalize latencies never gate the in-order PE stream.
  - Denominator = ones[128x128] @ exp(scores) -> [128,512] PSUM with the sum
    replicated across partitions; reciprocal on DVE (full tile, same cost as
    one row), then one tensor_tensor multiply. No gpsimd broadcast.
  - Score/psa/psD matmuls stream only the valid column range per key block
    (full-width block first with start=True so has_written covers the bank).
"""

import math
from contextlib import ExitStack

import numpy as np
import ml_dtypes

import concourse.bass as bass
import concourse.mybir as mybir
import concourse.tile as tile
from concourse import bacc
from concourse.bass_utils import run_bass_kernel_spmd

P = 128
S = 2048
E = 4096
D = 128
HPC = 4          # q heads per core
NE = E // P      # 32 e-tiles
NSCH = 4         # s-chunks of 512
SCH = S // NSCH  # 512
NST = S // P     # 16 s-tiles
NEO = 8          # output e-chunks of 512
SCALE = 1.0 / math.sqrt(D)
F16 = mybir.dt.float16
F32 = mybir.dt.float32
BF16 = mybir.dt.bfloat16

P1_ORDER = [1, 3, 0, 2]   # phase-1 chunk processing order
ATT_ORDER = [1, 3, 0, 2]  # attention chunk order (o-proj shifted one behind)


def _allowed_tiles(c):
    """For s-chunk c (query blocks bi=4c..4c+3), list (bj, lo, hi, mask, mpos):
    key tile bj is needed for query sub-tiles [lo, hi) (chunk-relative);
    mask in {None,'low','up'} applied at chunk-relative position mpos.
    Ordered with a full-width block first (for PSUM start=True coverage)."""
    out = []
    bis = range(4 * c, 4 * c + 4)
    for bj in range(NST):
        ok = [bi for bi in bis if abs(bi - bj) >= 8]
        if not ok:
            continue
        lo = min(ok) - 4 * c
        hi = max(ok) + 1 - 4 * c
        assert ok == list(range(lo + 4 * c, hi + 4 * c)), (c, bj, ok)
        mask, mpos = None, 0
        if bj - 8 in ok:
            mask, mpos = "low", bj - 8 - 4 * c
        elif bj + 8 in ok:
            mask, mpos = "up", bj + 8 - 4 * c
        out.append((bj, lo, hi, mask, mpos))
    out.sort(key=lambda t: (t[1] - t[2], t[0]))  # widest first
    assert out[0][1] == 0 and out[0][2] == 4, (c, out[0])
    return out


def build_nc():
    nc = bacc.Bacc("TRN2", target_bir_lowering=False, debug=False)
    hidT = nc.dram_tensor("hidT", (E, S), F16, kind="ExternalInput")
    wqT = nc.dram_tensor("wqT", (E, HPC * D), F16, kind="ExternalInput")
    wkT = nc.dram_tensor("wkT", (E, D), F16, kind="ExternalInput")
    wvT = nc.dram_tensor("wvT", (E, D), F16, kind="ExternalInput")
    woT = nc.dram_tensor("woT", (HPC * D, E), F16, kind="ExternalInput")
    cosT = nc.dram_tensor("cosT", (D, S), F16, kind="ExternalInput")
    sinT = nc.dram_tensor("sinT", (D, S), F16, kind="ExternalInput")
    # additive masks: -1e4 at banned positions of the boundary blocks, and an
    # identity used to add them to scores in PSUM via a 128-col matmul
    mlneg = nc.dram_tensor("mlneg", (P, P), F16, kind="ExternalInput")
    muneg = nc.dram_tensor("muneg", (P, P), F16, kind="ExternalInput")
    ident = nc.dram_tensor("ident", (P, P), F16, kind="ExternalInput")
    outd = nc.dram_tensor("out", (S, E), F16, kind="ExternalOutput")

    with tile.TileContext(nc) as tc, ExitStack() as ctx:
        const = ctx.enter_context(tc.tile_pool(name="const", bufs=1))

        wqT_r = wqT.rearrange("(eo p) d -> p eo d", p=P)
        wkT_r = wkT.rearrange("(eo p) d -> p eo d", p=P)
        wvT_r = wvT.rearrange("(eo p) d -> p eo d", p=P)
        hidT_r = hidT.rearrange("(eo p) s -> p eo s", p=P)
        woT_r = woT.rearrange("(ho p) e -> p ho e", p=P)

        NP = NE // 2  # e-tile pairs
        wq_t = [const.tile([P, 2, HPC * D], F16, name=f"wq{i}") for i in range(NP)]
        wk_t = [const.tile([P, 2, D], F16, name=f"wk{i}") for i in range(NP)]
        wv_t = [const.tile([P, 2, D], F16, name=f"wv{i}") for i in range(NP)]
        wo_t = [const.tile([P, E], F16, name=f"wo{h}") for h in range(HPC)]
        cos_sb = const.tile([P, S], F16)
        sin_sb = const.tile([P, S], F16)
        ml_sb = const.tile([P, P], F16)
        mu_sb = const.tile([P, P], F16)
        id_sb = const.tile([P, P], F16)
        ones_sb = const.tile([P, P], F16)
        nc.gpsimd.memset(ones_sb[:], 1.0)
        # Tiny dummy exp so the activation table load happens during the
        # initial DMA fill instead of gating the first attention block.
        warm = const.tile([1, 1], F32)
        nc.scalar.activation(
            warm[:], ones_sb[0:1, 0:1], mybir.ActivationFunctionType.Exp)

        qT_sb = const.tile([P, HPC, S], F16)     # Q^T per head [d, s]
        kT_sb = const.tile([P, S], F16)          # K^T [d, s]
        v_sb = const.tile([P, NST, D], F16)      # V [s-tile, d]
        attn_sb = const.tile([P, HPC, S], F16)   # attn_out^T per head [d, s]

        hidp = ctx.enter_context(tc.tile_pool(name="hid", bufs=4))
        rp = ctx.enter_context(tc.tile_pool(name="rope", bufs=2))

        def rope_drain(src_psum):
            raw = rp.tile([P, SCH], F16, tag="raw", bufs=6)
            nc.vector.tensor_copy(raw[:], src_psum)
            return raw

        def rope_apply(raw, dst_ap, c):
            rot = rp.tile([P, SCH], F16, tag="rot", bufs=2)
            nc.sync.dma_start(rot[0:64, :], raw[64:128, :])
            nc.sync.dma_start(rot[64:128, :], raw[0:64, :])
            t1 = rp.tile([P, SCH], F16, tag="t1", bufs=2)
            nc.vector.tensor_tensor(
                t1[:], raw[:], cos_sb[:, c * SCH:(c + 1) * SCH], mybir.AluOpType.mult)
            t2 = rp.tile([P, SCH], F16, tag="t2", bufs=2)
            nc.vector.tensor_tensor(
                t2[:], rot[:], sin_sb[:, c * SCH:(c + 1) * SCH], mybir.AluOpType.mult)
            nc.vector.tensor_tensor(dst_ap, t1[:], t2[:], mybir.AluOpType.add)

        # ---- Phase 1: QKV projections (+RoPE), inputs streamed in ----
        # Flat 64-step schedule (4 chunks x 16 e-tile pairs) with fixed DMA
        # lookahead so hid/weight transfers stay just ahead of the PE and
        # phase-boundary stalls vanish. Lower-priority inputs (cos/sin,
        # masks, wo) are slotted in after the critical stream.
        steps = [(c, i) for c in P1_ORDER for i in range(NP)]
        ht_tiles = {}

        def issue_ht(n):
            # scalar-queue HWDGE: the Sync DGE's ~0.7us/issue was co-critical
            # with compute in phase 1 when it carried both weights and hid
            c, i = steps[n]
            t = hidp.tile([P, 2, SCH], F16, tag="hid", bufs=6)
            nc.scalar.dma_start(
                t[:], hidT_r[:, 2 * i:2 * i + 2, c * SCH:(c + 1) * SCH])
            ht_tiles[n] = t

        def issue_w(i):
            nc.sync.dma_start(wq_t[i][:], wqT_r[:, 2 * i:2 * i + 2, :])
            nc.sync.dma_start(wk_t[i][:], wkT_r[:, 2 * i:2 * i + 2, :])
            nc.sync.dma_start(wv_t[i][:], wvT_r[:, 2 * i:2 * i + 2, :])

        with tc.tile_pool(name="p1psum", bufs=1, space="PSUM") as p1, \
             tc.tile_pool(name="p1kv", bufs=2, space="PSUM") as p1kv:
            issue_w(0)
            issue_ht(0)
            issue_ht(1)
            psq = psk = psvT = None
            for n, (c, i) in enumerate(steps):
                if i == 0:
                    psq = p1.tile([P, HPC, SCH], F32, tag="psq")   # 4 banks
                    psk = p1kv.tile([P, SCH], F32, tag="psk")      # 2 banks
                    psvT = p1kv.tile([P, SCH], F32, tag="psv")     # 2 banks
                ht = ht_tiles.pop(n)
                for t in range(2):
                    e = 2 * i + t
                    st = (e == 0)
                    sp = (e == NE - 1)
                    for h in range(HPC):
                        nc.tensor.matmul(
                            psq[:, h, :], wq_t[i][:, t, h * D:(h + 1) * D],
                            ht[:, t, :], start=st, stop=sp)
                    nc.tensor.matmul(
                        psk[:], wk_t[i][:, t, :], ht[:, t, :], start=st, stop=sp)
                    nc.tensor.matmul(
                        psvT[:], wv_t[i][:, t, :], ht[:, t, :], start=st, stop=sp)
                if n + 1 < NP:
                    issue_w(n + 1)
                if n + 2 < len(steps):
                    issue_ht(n + 2)
                if n == 8:
                    nc.sync.dma_start(cos_sb[:], cosT[:])
                    nc.sync.dma_start(sin_sb[:], sinT[:])
                elif n == 20:
                    nc.sync.dma_start(ml_sb[:], mlneg[:])
                    nc.sync.dma_start(mu_sb[:], muneg[:])
                    nc.sync.dma_start(id_sb[:], ident[:])
                elif 24 <= n < 24 + HPC:
                    nc.sync.dma_start(wo_t[n - 24][:], woT_r[:, n - 24, :])
                if i == NP - 1:
                    vstage = rp.tile([P, SCH], F16, tag="vstage", bufs=2)
                    nc.scalar.copy(vstage[:], psvT[:])
                    nc.sync.dma_start_transpose(
                        v_sb[:, c * 4:(c + 1) * 4, :], vstage[:])
                    kraw = rope_drain(psk[:])
                    # single wide CAST frees all 4 psq banks in one op, so
                    # the next chunk (or attention) reuses them ~2us sooner
                    qraw4 = rp.tile([P, HPC, SCH], F16, tag="qraw4", bufs=2)
                    nc.vector.tensor_copy(qraw4[:], psq[:])
                    rope_apply(kraw, kT_sb[:, c * SCH:(c + 1) * SCH], c)
                    for h in range(HPC):
                        rope_apply(qraw4[:, h, :],
                                   qT_sb[:, h, c * SCH:(c + 1) * SCH], c)

        # ---- Phase 2+3: attention, O projection one chunk behind ----
        ep = ctx.enter_context(tc.tile_pool(name="expp", bufs=3))
        np_pool = ctx.enter_context(tc.tile_pool(name="normp", bufs=2))
        osp = ctx.enter_context(tc.tile_pool(name="ostage", bufs=4))
        ap = ctx.enter_context(tc.tile_pool(name="apsum", bufs=2, space="PSUM"))

        def attention(c, h):
            blocks = _allowed_tiles(c)
            nblk = len(blocks)
            if True:
                psa = ap.tile([P, SCH], F32, tag="psa")
                psD = ap.tile([P, SCH], F32, tag="psd")
                ets = [None] * nblk
                # software-pipeline: scores/exp run one block ahead of psa/psD
                for idx in range(nblk + 1):
                    if idx < nblk:
                        bj, lo, hi, mask, mpos = blocks[idx]
                        n = (hi - lo) * P
                        pss = ap.tile([P, SCH], F32, tag="pss")
                        nc.tensor.matmul(
                            pss[:, :n],
                            kT_sb[:, bj * P:(bj + 1) * P],
                            qT_sb[:, h, c * SCH + lo * P: c * SCH + hi * P],
                            start=True, stop=(mask is None))
                        if mask is not None:
                            # add -1e4 at banned positions on the PE: identity
                            # stationary, pre-scaled triangle moving -> exp
                            # underflows to exact 0, no DVE op in the chain
                            m_sb = ml_sb if mask == "low" else mu_sb
                            mp = (mpos - lo) * P
                            nc.tensor.matmul(
                                pss[:, mp:mp + P], id_sb[:], m_sb[:],
                                start=False, stop=True)
                        et = ep.tile([P, SCH], BF16, tag="exp")
                        nc.scalar.activation(
                            et[:, lo * P:hi * P], pss[:, :n],
                            mybir.ActivationFunctionType.Exp, scale=SCALE)
                        ets[idx] = et
                    if idx >= 1:
                        bj, lo, hi, _, _ = blocks[idx - 1]
                        et = ets[idx - 1]
                        nc.tensor.matmul(
                            psa[:, lo * P:hi * P], v_sb[:, bj, :],
                            et[:, lo * P:hi * P],
                            start=(idx == 1), stop=(idx == nblk))
                        nc.tensor.matmul(
                            psD[:, lo * P:hi * P], ones_sb[:],
                            et[:, lo * P:hi * P],
                            start=(idx == 1), stop=(idx == nblk))
                return psa, psD

        def normalize_ops(c, h, psa, psD):
            # reciprocal is 8 cyc/elem on the DVE; split it (and the mult)
            # into small pieces so no single op blocks the in-order Vector
            # queue for more than ~0.9us (o-proj drains share that queue)
            invD = np_pool.tile([P, SCH], F32, tag="invd")
            ops = []
            for j in range(4):
                sl = slice(j * P, (j + 1) * P)
                ops.append(lambda sl=sl: nc.vector.reciprocal(
                    invD[:, sl], psD[:, sl]))
            for j in range(2):
                sl = slice(j * SCH // 2, (j + 1) * SCH // 2)
                osl = slice(c * SCH + j * SCH // 2,
                            c * SCH + (j + 1) * SCH // 2)
                ops.append(lambda sl=sl, osl=osl: nc.vector.tensor_tensor(
                    attn_sb[:, h, osl], psa[:, sl], invD[:, sl],
                    mybir.AluOpType.mult))
            return ops

        def o_proj_tile(st, extra_ops=()):
            extra_ops = list(extra_ops)
            orow = osp.tile([P, E], F16, tag="orow", bufs=2)
            for eo in range(NEO):
                pso = ap.tile([P, SCH], F32, tag="pso")
                for h in range(HPC):
                    nc.tensor.matmul(
                        pso[:],
                        attn_sb[:, h, st * P:(st + 1) * P],
                        wo_t[h][:, eo * SCH:(eo + 1) * SCH],
                        start=(h == 0), stop=(h == HPC - 1))
                nc.vector.tensor_copy(
                    orow[:, eo * SCH:(eo + 1) * SCH], pso[:])
                if extra_ops:
                    extra_ops.pop(0)()
                if eo == NEO // 2 - 1:
                    nc.sync.dma_start(
                        outd[st * P:(st + 1) * P, :E // 2],
                        orow[:, :E // 2])
            nc.sync.dma_start(
                outd[st * P:(st + 1) * P, E // 2:], orow[:, E // 2:])
            for op in extra_ops:
                op()

        # attention heads of chunk c interleave 1:1 with O-projection
        # s-tiles of the previous chunk. Exps (Scalar) gate psa matmuls and
        # drains (Vector) gate pso reuse, so each engine carries only its
        # PE-gating op stream; recip+mult trail at the end of each pair
        # (they only gate the NEXT chunk's O projection).
        prev = None
        for c in ATT_ORDER:
            for h in range(HPC):
                psa, psD = attention(c, h)
                ops = normalize_ops(c, h, psa, psD)
                if prev is not None:
                    o_proj_tile(4 * prev + h, ops)
                else:
                    for op in ops:
                        op()
            prev = c
        for h in range(HPC):
            o_proj_tile(4 * prev + h)
    nc.compile()
    return nc


_NC_CACHE = {}


def get_nc():
    if "nc" not in _NC_CACHE:
        _NC_CACHE["nc"] = build_nc()
    return _NC_CACHE["nc"]


def make_in_maps(hidden_states, Wq, Wk, Wv, Wo):
    hid = np.asarray(hidden_states).reshape(S, E)
    hidT16 = np.ascontiguousarray(hid.T).astype(np.float16)

    inv = 1.0 / (10000.0 ** (np.arange(0, D, 2, dtype=np.float64) / D))
    t = np.arange(S, dtype=np.float64)
    fr = np.outer(t, inv)                      # [S, 64]
    emb = np.concatenate([fr, fr], axis=1)     # [S, 128]
    cosT = np.ascontiguousarray(np.cos(emb).T).astype(np.float16)
    sinT = np.ascontiguousarray(np.sin(emb).T).astype(np.float16)
    sinT[:64] *= -1.0                          # rotate_half sign fold

    jj = np.arange(P)[:, None]
    ii = np.arange(P)[None, :]
    # additive -1e4 at banned positions (block bj-bi=8 keeps j-i>=1024;
    # block bi-bj=8 keeps i-j>=1024)
    mlneg = (-1e4 * (jj < ii)).astype(np.float16)
    muneg = (-1e4 * (ii < jj)).astype(np.float16)
    ident = np.eye(P, dtype=np.float16)

    in_maps = []
    for c in range(8):
        qsl = slice(c * 512, (c + 1) * 512)
        ksl = slice(c * 128, (c + 1) * 128)
        in_maps.append({
            "hidT": hidT16,
            "wqT": np.ascontiguousarray(Wq[qsl].T).astype(np.float16),
            "wkT": np.ascontiguousarray(Wk[ksl].T).astype(np.float16),
            "wvT": np.ascontiguousarray(Wv[ksl].T).astype(np.float16),
            "woT": np.ascontiguousarray(Wo[:, qsl].T).astype(np.float16),
            "cosT": cosT,
            "sinT": sinT,
            "mlneg": mlneg,
            "muneg": muneg,
            "ident": ident,
        })
    return in_maps


def run(in_maps, **kwargs):
    nc = get_nc()
    return run_bass_kernel_spmd(nc, in_maps, core_ids=list(range(8)), **kwargs)


def kernel(hidden_states, Wq, Wk, Wv, Wo):
    in_maps = make_in_maps(hidden_states, Wq, Wk, Wv, Wo)
    res = run(in_maps)
    out = np.zeros((S, E), dtype=np.float32)
    for r in res.results:
        out += r["out"].astype(np.float32)
    return out.reshape(1, S, E)


# revision 34
# speedup vs baseline: 1.0871x; 1.0271x over previous
"""Trainium2 Bass kernel for Mistral-style attention with an INVERTED band mask.

Reference semantics (S=2048, E=4096, H=32, KV=8, D=128, WINDOW=1024):
  q/k/v projections -> RoPE(q,k) -> GQA attention where positions with
  |i-j| < 1024 are masked OUT (attend only to far positions) -> softmax ->
  out projection.

Sharding (8 cores, tensor-parallel by GQA group):
  core c owns KV head c and Q heads 4c..4c+3. Column-parallel QKV,
  row-parallel O projection; the 8 fp16 partial outputs are summed on host.

On-device layout: everything transposed so matmuls contract on partitions.
  Host passes hidden^T, Wq^T/Wk^T/Wv^T slices, Wo^T slice, RoPE tables
  (transposed, sign-folded, fp16), and two 128x128 triangular masks for the
  blocks that straddle the |i-j|=1024 boundary.

Schedule notes (v2):
  - Input DMAs are interleaved with phase-1 matmuls (weights stream in per
    e-tile pair) so the PE starts ~2us in instead of after the full 29MB load.
  - Phase-1 chunks are processed in order [1,3,0,2] and attention chunks in
    [1,3,0,2] with the O projection shifted one chunk behind, so RoPE/exp/
    normalize latencies never gate the in-order PE stream.
  - Denominator = ones[128x128] @ exp(scores) -> [128,512] PSUM with the sum
    replicated across partitions; reciprocal on DVE (full tile, same cost as
    one row), then one tensor_tensor multiply. No gpsimd broadcast.
  - Score/psa/psD matmuls stream only the valid column range per key block
    (full-width block first with start=True so has_written covers the bank).
"""

import math
from contextlib import ExitStack

import numpy as np
import ml_dtypes

import concourse.bass as bass
import concourse.mybir as mybir
import concourse.tile as tile
from concourse import bacc
from concourse.bass_utils import run_bass_kernel_spmd

P = 128
S = 2048
E = 4096
D = 128
HPC = 4          # q heads per core
NE = E // P      # 32 e-tiles
NSCH = 4         # s-chunks of 512
SCH = S // NSCH  # 512
NST = S // P     # 16 s-tiles
NEO = 8          # output e-chunks of 512
SCALE = 1.0 / math.sqrt(D)
F16 = mybir.dt.float16
F32 = mybir.dt.float32
BF16 = mybir.dt.bfloat16

P1_ORDER = [1, 3, 0, 2]   # phase-1 chunk processing order
ATT_ORDER = [1, 3, 0, 2]  # attention chunk order (o-proj shifted one behind)


def _allowed_tiles(c):
    """For s-chunk c (query blocks bi=4c..4c+3), list (bj, lo, hi, mask, mpos):
    key tile bj is needed for query sub-tiles [lo, hi) (chunk-relative);
    mask in {None,'low','up'} applied at chunk-relative position mpos.
    Ordered with a full-width block first (for PSUM start=True coverage)."""
    out = []
    bis = range(4 * c, 4 * c + 4)
    for bj in range(NST):
        ok = [bi for bi in bis if abs(bi - bj) >= 8]
        if not ok:
            continue
        lo = min(ok) - 4 * c
        hi = max(ok) + 1 - 4 * c
        assert ok == list(range(lo + 4 * c, hi + 4 * c)), (c, bj, ok)
        mask, mpos = None, 0
        if bj - 8 in ok:
            mask, mpos = "low", bj - 8 - 4 * c
        elif bj + 8 in ok:
            mask, mpos = "up", bj + 8 - 4 * c
        out.append((bj, lo, hi, mask, mpos))
    out.sort(key=lambda t: (t[1] - t[2], t[0]))  # widest first
    assert out[0][1] == 0 and out[0][2] == 4, (c, out[0])
    return out


def build_nc():
    nc = bacc.Bacc("TRN2", target_bir_lowering=False, debug=False)
    hidT = nc.dram_tensor("hidT", (E, S), F16, kind="ExternalInput")
    wqT = nc.dram_tensor("wqT", (E, HPC * D), F16, kind="ExternalInput")
    wkT = nc.dram_tensor("wkT", (E, D), F16, kind="ExternalInput")
    wvT = nc.dram_tensor("wvT", (E, D), F16, kind="ExternalInput")
    woT = nc.dram_tensor("woT", (HPC * D, E), F16, kind="ExternalInput")
    cosT = nc.dram_tensor("cosT", (D, S), F16, kind="ExternalInput")
    sinT = nc.dram_tensor("sinT", (D, S), F16, kind="ExternalInput")
    # additive masks: -1e4 at banned positions of the boundary blocks, and an
    # identity used to add them to scores in PSUM via a 128-col matmul
    mlneg = nc.dram_tensor("mlneg", (P, P), F16, kind="ExternalInput")
    muneg = nc.dram_tensor("muneg", (P, P), F16, kind="ExternalInput")
    ident = nc.dram_tensor("ident", (P, P), F16, kind="ExternalInput")
    outd = nc.dram_tensor("out", (S, E), F16, kind="ExternalOutput")

    with tile.TileContext(nc) as tc, ExitStack() as ctx:
        const = ctx.enter_context(tc.tile_pool(name="const", bufs=1))

        wqT_r = wqT.rearrange("(eo p) d -> p eo d", p=P)
        wkT_r = wkT.rearrange("(eo p) d -> p eo d", p=P)
        wvT_r = wvT.rearrange("(eo p) d -> p eo d", p=P)
        hidT_r = hidT.rearrange("(eo p) s -> p eo s", p=P)
        woT_r = woT.rearrange("(ho p) e -> p ho e", p=P)

        NP = NE // 2  # e-tile pairs
        wq_t = [const.tile([P, 2, HPC * D], F16, name=f"wq{i}") for i in range(NP)]
        wk_t = [const.tile([P, 2, D], F16, name=f"wk{i}") for i in range(NP)]
        wv_t = [const.tile([P, 2, D], F16, name=f"wv{i}") for i in range(NP)]
        wo_t = [const.tile([P, E], F16, name=f"wo{h}") for h in range(HPC)]
        cos_sb = const.tile([P, S], F16)
        sin_sb = const.tile([P, S], F16)
        ml_sb = const.tile([P, P], F16)
        mu_sb = const.tile([P, P], F16)
        id_sb = const.tile([P, P], F16)
        ones_sb = const.tile([P, P], F16)
        nc.gpsimd.memset(ones_sb[:], 1.0)
        # Tiny dummy exp so the activation table load happens during the
        # initial DMA fill instead of gating the first attention block.
        warm = const.tile([1, 1], F32)
        nc.scalar.activation(
            warm[:], ones_sb[0:1, 0:1], mybir.ActivationFunctionType.Exp)

        qT_sb = const.tile([P, HPC, S], F16)     # Q^T per head [d, s]
        kT_sb = const.tile([P, S], F16)          # K^T [d, s]
        v_sb = const.tile([P, NST, D], F16)      # V [s-tile, d]
        attn_sb = const.tile([P, HPC, S], F16)   # attn_out^T per head [d, s]

        hidp = ctx.enter_context(tc.tile_pool(name="hid", bufs=4))
        rp = ctx.enter_context(tc.tile_pool(name="rope", bufs=2))

        def rope_drain(src_psum):
            raw = rp.tile([P, SCH], F16, tag="raw", bufs=6)
            nc.vector.tensor_copy(raw[:], src_psum)
            return raw

        def rope_apply(raw, dst_ap, c):
            rot = rp.tile([P, SCH], F16, tag="rot", bufs=2)
            nc.sync.dma_start(rot[0:64, :], raw[64:128, :])
            nc.sync.dma_start(rot[64:128, :], raw[0:64, :])
            t1 = rp.tile([P, SCH], F16, tag="t1", bufs=2)
            nc.vector.tensor_tensor(
                t1[:], raw[:], cos_sb[:, c * SCH:(c + 1) * SCH], mybir.AluOpType.mult)
            t2 = rp.tile([P, SCH], F16, tag="t2", bufs=2)
            nc.vector.tensor_tensor(
                t2[:], rot[:], sin_sb[:, c * SCH:(c + 1) * SCH], mybir.AluOpType.mult)
            nc.vector.tensor_tensor(dst_ap, t1[:], t2[:], mybir.AluOpType.add)

        # ---- Phase 1: QKV projections (+RoPE), inputs streamed in ----
        # Flat 64-step schedule (4 chunks x 16 e-tile pairs) with fixed DMA
        # lookahead so hid/weight transfers stay just ahead of the PE and
        # phase-boundary stalls vanish. Lower-priority inputs (cos/sin,
        # masks, wo) are slotted in after the critical stream.
        steps = [(c, i) for c in P1_ORDER for i in range(NP)]
        ht_tiles = {}

        def issue_ht(n):
            # scalar-queue HWDGE: the Sync DGE's ~0.7us/issue was co-critical
            # with compute in phase 1 when it carried both weights and hid
            c, i = steps[n]
            t = hidp.tile([P, 2, SCH], F16, tag="hid", bufs=6)
            nc.scalar.dma_start(
                t[:], hidT_r[:, 2 * i:2 * i + 2, c * SCH:(c + 1) * SCH])
            ht_tiles[n] = t

        def issue_w(i):
            nc.sync.dma_start(wq_t[i][:], wqT_r[:, 2 * i:2 * i + 2, :])
            nc.sync.dma_start(wk_t[i][:], wkT_r[:, 2 * i:2 * i + 2, :])
            nc.sync.dma_start(wv_t[i][:], wvT_r[:, 2 * i:2 * i + 2, :])

        # p1kv first: its banks drain early (psk/vstage), so the attention
        # pool's first tiles (pss/psa) recycle them with less waiting
        with tc.tile_pool(name="p1kv", bufs=2, space="PSUM") as p1kv, \
             tc.tile_pool(name="p1psum", bufs=1, space="PSUM") as p1:
            issue_w(0)
            issue_ht(0)
            issue_ht(1)
            issue_ht(2)
            psq = psk = psvT = None
            for n, (c, i) in enumerate(steps):
                if i == 0:
                    psq = p1.tile([P, HPC, SCH], F32, tag="psq")   # 4 banks
                    psk = p1kv.tile([P, SCH], F32, tag="psk")      # 2 banks
                    psvT = p1kv.tile([P, SCH], F32, tag="psv")     # 2 banks
                ht = ht_tiles.pop(n)
                for t in range(2):
                    e = 2 * i + t
                    st = (e == 0)
                    sp = (e == NE - 1)
                    for h in range(HPC):
                        nc.tensor.matmul(
                            psq[:, h, :], wq_t[i][:, t, h * D:(h + 1) * D],
                            ht[:, t, :], start=st, stop=sp)
                    nc.tensor.matmul(
                        psk[:], wk_t[i][:, t, :], ht[:, t, :], start=st, stop=sp)
                    nc.tensor.matmul(
                        psvT[:], wv_t[i][:, t, :], ht[:, t, :], start=st, stop=sp)
                if n + 1 < NP:
                    issue_w(n + 1)
                if n + 3 < len(steps):
                    issue_ht(n + 3)
                if n == 14:
                    # needed by chunk-1 rope (DVE, off the PE critical path);
                    # issuing later keeps the pre-boundary HBM window for
                    # weights+hid, which bind at ~395GB/s demand
                    nc.sync.dma_start(cos_sb[:], cosT[:])
                    nc.sync.dma_start(sin_sb[:], sinT[:])
                elif n == 20:
                    nc.sync.dma_start(ml_sb[:], mlneg[:])
                    nc.sync.dma_start(mu_sb[:], muneg[:])
                    nc.sync.dma_start(id_sb[:], ident[:])
                elif 24 <= n < 24 + HPC:
                    nc.sync.dma_start(wo_t[n - 24][:], woT_r[:, n - 24, :])
                if i == NP - 1:
                    vstage = rp.tile([P, SCH], F16, tag="vstage", bufs=2)
                    nc.scalar.copy(vstage[:], psvT[:])
                    nc.sync.dma_start_transpose(
                        v_sb[:, c * 4:(c + 1) * 4, :], vstage[:])
                    kraw = rope_drain(psk[:])
                    # single wide CAST frees all 4 psq banks in one op, so
                    # the next chunk (or attention) reuses them ~2us sooner
                    qraw4 = rp.tile([P, HPC, SCH], F16, tag="qraw4", bufs=2)
                    nc.vector.tensor_copy(qraw4[:], psq[:])
                    rope_apply(kraw, kT_sb[:, c * SCH:(c + 1) * SCH], c)
                    for h in range(HPC):
                        rope_apply(qraw4[:, h, :],
                                   qT_sb[:, h, c * SCH:(c + 1) * SCH], c)

        # ---- Phase 2+3: attention, O projection one chunk behind ----
        ep = ctx.enter_context(tc.tile_pool(name="expp", bufs=3))
        np_pool = ctx.enter_context(tc.tile_pool(name="normp", bufs=2))
        osp = ctx.enter_context(tc.tile_pool(name="ostage", bufs=4))
        ap = ctx.enter_context(tc.tile_pool(name="apsum", bufs=2, space="PSUM"))

        def attention(c, h):
            blocks = _allowed_tiles(c)
            nblk = len(blocks)
            if True:
                psa = ap.tile([P, SCH], F32, tag="psa")
                psD = ap.tile([P, SCH], F32, tag="psd")
                ets = [None] * nblk
                # software-pipeline: scores/exp run one block ahead of psa/psD
                for idx in range(nblk + 1):
                    if idx < nblk:
                        bj, lo, hi, mask, mpos = blocks[idx]
                        n = (hi - lo) * P
                        pss = ap.tile([P, SCH], F32, tag="pss")
                        nc.tensor.matmul(
                            pss[:, :n],
                            kT_sb[:, bj * P:(bj + 1) * P],
                            qT_sb[:, h, c * SCH + lo * P: c * SCH + hi * P],
                            start=True, stop=(mask is None))
                        if mask is not None:
                            # add -1e4 at banned positions on the PE: identity
                            # stationary, pre-scaled triangle moving -> exp
                            # underflows to exact 0, no DVE op in the chain
                            m_sb = ml_sb if mask == "low" else mu_sb
                            mp = (mpos - lo) * P
                            nc.tensor.matmul(
                                pss[:, mp:mp + P], id_sb[:], m_sb[:],
                                start=False, stop=True)
                        et = ep.tile([P, SCH], BF16, tag="exp")
                        nc.scalar.activation(
                            et[:, lo * P:hi * P], pss[:, :n],
                            mybir.ActivationFunctionType.Exp, scale=SCALE)
                        ets[idx] = et
                    if idx >= 1:
                        bj, lo, hi, _, _ = blocks[idx - 1]
                        et = ets[idx - 1]
                        nc.tensor.matmul(
                            psa[:, lo * P:hi * P], v_sb[:, bj, :],
                            et[:, lo * P:hi * P],
                            start=(idx == 1), stop=(idx == nblk))
                        nc.tensor.matmul(
                            psD[:, lo * P:hi * P], ones_sb[:],
                            et[:, lo * P:hi * P],
                            start=(idx == 1), stop=(idx == nblk))
                return psa, psD

        def normalize_ops(c, h, psa, psD):
            # reciprocal is 8 cyc/elem on the DVE; split it (and the mult)
            # into small pieces so no single op blocks the in-order Vector
            # queue for more than ~0.9us (o-proj drains share that queue)
            invD = np_pool.tile([P, SCH], F32, tag="invd")
            ops = []
            for j in range(4):
                sl = slice(j * P, (j + 1) * P)
                ops.append(lambda sl=sl: nc.vector.reciprocal(
                    invD[:, sl], psD[:, sl]))
            for j in range(2):
                sl = slice(j * SCH // 2, (j + 1) * SCH // 2)
                osl = slice(c * SCH + j * SCH // 2,
                            c * SCH + (j + 1) * SCH // 2)
                ops.append(lambda sl=sl, osl=osl: nc.vector.tensor_tensor(
                    attn_sb[:, h, osl], psa[:, sl], invD[:, sl],
                    mybir.AluOpType.mult))
            return ops

        def o_proj_tile(st, extra_ops=(), finegrain_out=False):
            extra_ops = list(extra_ops)
            orow = osp.tile([P, E], F16, tag="orow", bufs=2)
            # on the kernel's final tile, ship output per eo-pair so the
            # last DMA is 0.25MB instead of 2MB
            out_step = 2 if finegrain_out else NEO // 2
            for eo in range(NEO):
                pso = ap.tile([P, SCH], F32, tag="pso")
                for h in range(HPC):
                    nc.tensor.matmul(
                        pso[:],
                        attn_sb[:, h, st * P:(st + 1) * P],
                        wo_t[h][:, eo * SCH:(eo + 1) * SCH],
                        start=(h == 0), stop=(h == HPC - 1))
                nc.vector.tensor_copy(
                    orow[:, eo * SCH:(eo + 1) * SCH], pso[:])
                if extra_ops:
                    extra_ops.pop(0)()
                if eo % out_step == out_step - 1 and eo != NEO - 1:
                    lo, hi = (eo + 1 - out_step) * SCH, (eo + 1) * SCH
                    nc.sync.dma_start(
                        outd[st * P:(st + 1) * P, lo:hi], orow[:, lo:hi])
            lo = (NEO - out_step) * SCH
            nc.sync.dma_start(
                outd[st * P:(st + 1) * P, lo:], orow[:, lo:])
            for op in extra_ops:
                op()

        # attention heads of chunk c interleave 1:1 with O-projection
        # s-tiles of the previous chunk. Exps (Scalar) gate psa matmuls and
        # drains (Vector) gate pso reuse, so each engine carries only its
        # PE-gating op stream; recip+mult trail at the end of each pair
        # (they only gate the NEXT chunk's O projection).
        prev = None
        for c in ATT_ORDER:
            for h in range(HPC):
                psa, psD = attention(c, h)
                ops = normalize_ops(c, h, psa, psD)
                if prev is not None:
                    o_proj_tile(4 * prev + h, ops)
                else:
                    for op in ops:
                        op()
            prev = c
        for h in range(HPC):
            o_proj_tile(4 * prev + h, finegrain_out=(h == HPC - 1))
    nc.compile()
    return nc


_NC_CACHE = {}


def get_nc():
    if "nc" not in _NC_CACHE:
        _NC_CACHE["nc"] = build_nc()
    return _NC_CACHE["nc"]


def make_in_maps(hidden_states, Wq, Wk, Wv, Wo):
    hid = np.asarray(hidden_states).reshape(S, E)
    hidT16 = np.ascontiguousarray(hid.T).astype(np.float16)

    inv = 1.0 / (10000.0 ** (np.arange(0, D, 2, dtype=np.float64) / D))
    t = np.arange(S, dtype=np.float64)
    fr = np.outer(t, inv)                      # [S, 64]
    emb = np.concatenate([fr, fr], axis=1)     # [S, 128]
    cosT = np.ascontiguousarray(np.cos(emb).T).astype(np.float16)
    sinT = np.ascontiguousarray(np.sin(emb).T).astype(np.float16)
    sinT[:64] *= -1.0                          # rotate_half sign fold

    jj = np.arange(P)[:, None]
    ii = np.arange(P)[None, :]
    # additive -1e4 at banned positions (block bj-bi=8 keeps j-i>=1024;
    # block bi-bj=8 keeps i-j>=1024)
    mlneg = (-1e4 * (jj < ii)).astype(np.float16)
    muneg = (-1e4 * (ii < jj)).astype(np.float16)
    ident = np.eye(P, dtype=np.float16)

    in_maps = []
    for c in range(8):
        qsl = slice(c * 512, (c + 1) * 512)
        ksl = slice(c * 128, (c + 1) * 128)
        in_maps.append({
            "hidT": hidT16,
            "wqT": np.ascontiguousarray(Wq[qsl].T).astype(np.float16),
            "wkT": np.ascontiguousarray(Wk[ksl].T).astype(np.float16),
            "wvT": np.ascontiguousarray(Wv[ksl].T).astype(np.float16),
            "woT": np.ascontiguousarray(Wo[:, qsl].T).astype(np.float16),
            "cosT": cosT,
            "sinT": sinT,
            "mlneg": mlneg,
            "muneg": muneg,
            "ident": ident,
        })
    return in_maps


def run(in_maps, **kwargs):
    nc = get_nc()
    return run_bass_kernel_spmd(nc, in_maps, core_ids=list(range(8)), **kwargs)


def kernel(hidden_states, Wq, Wk, Wv, Wo):
    in_maps = make_in_maps(hidden_states, Wq, Wk, Wv, Wo)
    res = run(in_maps)
    out = np.zeros((S, E), dtype=np.float32)
    for r in res.results:
        out += r["out"].astype(np.float32)
    return out.reshape(1, S, E)
